# revision 2
# baseline (speedup 1.0000x reference)
import sys

sys.path.insert(0, "/opt/trn_rl_repo")
import numpy as np
from contextlib import ExitStack

from concourse import bacc
import concourse.tile as tile
from concourse import mybir
from concourse.bass_utils import run_bass_kernel_spmd

fp32 = mybir.dt.float32
bf16 = mybir.dt.bfloat16
Exp = mybir.ActivationFunctionType.Exp

B, S, HID = 4, 2048, 1024
H, DK = 16, 64
SK = 1152          # compacted+padded key count (max kept keys = 1036)
SKT = SK // 128    # 9 sk tiles
NPAIR = 4          # head pairs per core (8 heads = half the 16)

# blob column offsets (bf16 words per partition), in DMA/first-use order
OWK = 0
OXKV = OWK + 4096
OWV = OXKV + SK * 8
OWQ = OWV + 4096
OXQ = OWQ + 4096
OWO = OXQ + 16384
OMSK = OWO + 4096
BLOBW = OMSK + 16

_PROG = None


def _build_program():
    nc = bacc.Bacc("TRN2", target_bir_lowering=False)

    blob = nc.dram_tensor("blob", [128, BLOBW], bf16, kind="ExternalInput")
    y = nc.dram_tensor("y", [S, HID], fp32, kind="ExternalOutput")

    # SBUF arena (bf16 word offsets per partition), everything resident:
    #   KT   [0..4608)       K^T pair-major: KT[p, pair*1152 + sk]
    #   QT   [4608..12800)   Q^T: QT[p, pair*2048 + sq]
    #   YPN  [12800..20992)  normalized attn out^T: YPN[p, pair*2048 + sq]
    #   VP   [20992..30208)  pair*2304 + st*256 + [Va(64)|ma(64)|Vb(64)|mb(64)]
    #   WK   [30208..34304)  c-major weight chunks
    #   XKV  [34304..43520)  c-major: [:, c*1152 + sk]
    #   WV   [43520..47616)
    #   WQ   [47616..51712)
    #   XQ   [51712..68096)  q-major quarters, c-major within: q*4096 + c*512
    #   WO   [68096..72192)
    arena = nc.alloc_sbuf_tensor("arena", [128, 72192], bf16)
    base = nc.lookup_mloc(arena).addr

    def at(name, words, off_words):
        return nc.alloc_sbuf_tensor_at(
            name, [128, words], bf16, offset=base + off_words * 2
        )

    KT = at("KT", 4608, 0)
    QT = at("QT", 8192, 4608)
    YPN = at("YPN", 8192, 12800)
    VP = at("VP", 9216, 20992)
    WKs = at("WKs", 4096, 30208)
    XKVs = at("XKVs", 9216, 34304)
    WVs = at("WVs", 4096, 43520)
    WQs = at("WQs", 4096, 47616)
    XQs = at("XQs", 16384, 51712)
    WOs = at("WOs", 4096, 68096)

    with tile.TileContext(nc) as tc, ExitStack() as ctx:
        misc = ctx.enter_context(tc.tile_pool(name="misc", bufs=1))
        pt_pool = ctx.enter_context(tc.tile_pool(name="ptp", bufs=3))
        ev_pool = ctx.enter_context(tc.tile_pool(name="evp", bufs=3))
        rc_pool = ctx.enter_context(tc.tile_pool(name="rcp", bufs=2))
        ps_e = ctx.enter_context(tc.tile_pool(name="pse", bufs=3, space="PSUM"))
        ps_y = ctx.enter_context(tc.tile_pool(name="psy", bufs=2, space="PSUM"))

        masktb = misc.tile([128, 16], bf16)
        nc.sync.dma_start(masktb[:], blob[:, OMSK:OMSK + 16])
        maskt = misc.tile([128, 16], fp32)
        nc.vector.tensor_copy(maskt[:], masktb[:])

        # input stream from the blob, sliced by first use and split across
        # both HWDGE rings (sync + scalar) so transfers overlap compute
        nc.sync.dma_start(WKs[:], blob[:, OWK:OWK + 4096])
        for c in range(8):
            nc.sync.dma_start(
                XKVs[:, c * SK: c * SK + 384],
                blob[:, OXKV + c * SK: OXKV + c * SK + 384])
        for c in range(8):
            nc.scalar.dma_start(
                XKVs[:, c * SK + 384: c * SK + 768],
                blob[:, OXKV + c * SK + 384: OXKV + c * SK + 768])
        nc.sync.dma_start(WVs[:], blob[:, OWV:OWV + 4096])
        for c in range(8):
            nc.sync.dma_start(
                XKVs[:, c * SK + 768:(c + 1) * SK],
                blob[:, OXKV + c * SK + 768: OXKV + (c + 1) * SK])
        nc.scalar.dma_start(WQs[:], blob[:, OWQ:OWQ + 4096])
        nc.sync.dma_start(XQs[:, 0:4096], blob[:, OXQ:OXQ + 4096])
        nc.scalar.dma_start(XQs[:, 4096:8192],
                            blob[:, OXQ + 4096:OXQ + 8192])
        nc.sync.dma_start(XQs[:, 8192:12288],
                          blob[:, OXQ + 8192:OXQ + 12288])
        nc.scalar.dma_start(XQs[:, 12288:16384],
                            blob[:, OXQ + 12288:OXQ + 16384])
        nc.scalar.dma_start(WOs[:], blob[:, OWO:OWO + 4096])

        ones = misc.tile([128, 64], bf16)
        nc.vector.memset(ones[:], 1.0)

        # ---- Phase A: K^T -> KT, V (masked) -> VP ----
        def ktr_chunk(p, off, w):
            pk = ps_y.tile([128, 512], fp32, name="psyt")
            for c in range(8):
                nc.tensor.matmul(
                    pk[:, 0:w],
                    WKs[:, c * 512 + p * 128: c * 512 + (p + 1) * 128],
                    XKVs[:, c * SK + off: c * SK + off + w],
                    start=(c == 0), stop=(c == 7))
            nc.vector.tensor_copy(
                KT[:, p * SK + off: p * SK + off + w], pk[:, 0:w])

        def v_tile(st):
            pv = ps_e.tile([128, 1024], fp32, name="pe")
            for c in range(8):
                nc.tensor.matmul(
                    pv[:, 0:512],
                    XKVs[:, c * SK + st * 128: c * SK + (st + 1) * 128],
                    WVs[:, c * 512:(c + 1) * 512],
                    start=(c == 0), stop=(c == 7))
            for h in range(8):
                o = (h // 2) * 2304 + st * 256 + (h % 2) * 128
                nc.vector.tensor_scalar_mul(
                    VP[:, o:o + 64], pv[:, h * 64:(h + 1) * 64],
                    maskt[:, st:st + 1])

        for p in range(NPAIR):
            ktr_chunk(p, 0, 384)
        for p in range(NPAIR):
            ktr_chunk(p, 384, 384)
        for st in range(6):
            v_tile(st)
        for p in range(NPAIR):
            ktr_chunk(p, 768, 384)
        for st in range(6, SKT):
            v_tile(st)

        # ---- Phase B seed: Q^T for query block q0, all pairs ----
        def b_group(q, p):
            pq = ps_y.tile([128, 512], fp32, name="psyt")
            for c in range(8):
                nc.tensor.matmul(
                    pq[:],
                    WQs[:, c * 512 + p * 128: c * 512 + (p + 1) * 128],
                    XQs[:, q * 4096 + c * 512: q * 4096 + (c + 1) * 512],
                    start=(c == 0), stop=(c == 7))
            nc.vector.tensor_copy(
                QT[:, p * 2048 + q * 512: p * 2048 + (q + 1) * 512], pq[:])

        for p in range(NPAIR):
            b_group(0, p)

        # init VP mask columns (denominator ones, masked); stage-1 pairs
        # (2,3) first so the first combos' A*V reads are ready in time
        for p in (2, 3, 0, 1):
            for hh in range(2):
                for st in range(SKT):
                    o = p * 2304 + st * 256 + hh * 128 + 64
                    nc.vector.tensor_scalar_mul(
                        VP[:, o:o + 64], ones[:], maskt[:, st:st + 1])

        # ---- Phases C (attention) and D (out-proj), B fillers inside C ----
        def d_group(m, no):
            pd = ps_y.tile([128, 512], fp32, name="psyt")
            for tt in range(NPAIR):
                nc.tensor.matmul(
                    pd[:],
                    YPN[:, tt * 2048 + m * 128: tt * 2048 + (m + 1) * 128],
                    WOs[:, tt * 1024 + no * 512: tt * 1024 + no * 512 + 512],
                    start=(tt == 0), stop=(tt == 3))
            ob = ev_pool.tile([128, 512], fp32)
            nc.vector.tensor_copy(ob[:], pd[:])
            nc.sync.dma_start(
                y[m * 128:(m + 1) * 128, no * 512: no * 512 + 512], ob[:])

        NST = (2, 2, 2, 2, 1)  # sk tiles per exp stage (9 total)

        def c_combo(n, p, hh, host=None):
            # software-pipelined: pe/exp run 3 stages ahead of the A*V
            # matmuls; hosted filler/out-proj work runs in the exp ramp so
            # PE never waits on ACT.  py is allocated AFTER host() so the
            # 2-buf psum rotation never recycles an open accumulator.
            qsl = QT[hh * 64:(hh + 1) * 64,
                     p * 2048 + n * 512: p * 2048 + n * 512 + 512]
            pts = []

            def emit_pe(k):
                pe = ps_e.tile([128, 1024], fp32)
                w = 512 * NST[k]
                for j in range(NST[k]):
                    st = 2 * k + j
                    nc.tensor.matmul(
                        pe[:, j * 512:(j + 1) * 512],
                        KT[hh * 64:(hh + 1) * 64,
                           p * SK + st * 128: p * SK + (st + 1) * 128],
                        qsl, start=True, stop=True,
                        tile_position=(hh * 64, 0))
                pt = pt_pool.tile([128, 1024], bf16)
                nc.scalar.activation(pt[:, 0:w], pe[:, 0:w], Exp, scale=0.125)
                pts.append(pt)

            def emit_py(k, py):
                for j in range(NST[k]):
                    st = 2 * k + j
                    nc.tensor.matmul(
                        py[:],
                        VP[:, p * 2304 + st * 256 + hh * 128:
                           p * 2304 + st * 256 + hh * 128 + 128],
                        pts[k][:, j * 512:(j + 1) * 512],
                        start=(st == 0), stop=(st == SKT - 1))

            for k in range(3):
                emit_pe(k)
            if host is not None:
                host()
            py = ps_y.tile([128, 512], fp32, name="psyt")
            for k in range(3, 5):
                emit_py(k - 3, py)
                emit_pe(k)
            for k in range(2, 5):
                emit_py(k, py)
            rc = rc_pool.tile([64, 512], fp32)
            nc.vector.reciprocal(rc[:], py[64:128, :])
            nc.vector.tensor_mul(
                YPN[hh * 64:(hh + 1) * 64,
                    p * 2048 + n * 512: p * 2048 + n * 512 + 512],
                py[0:64, :], rc[:])

        # B fillers for stage 1: one group behind each combo; each (q,p)
        # filler precedes the first stage-1 combo reading QT(q,p).
        fillers = [(1, 2), (1, 3), (2, 2), (2, 3), (1, 0), (1, 1),
                   (2, 0), (3, 2), (3, 3), (2, 1), (3, 0), (3, 1)]

        def filler_host(i):
            def host():
                fq, fp = fillers[i]
                b_group(fq, fp)
            return host

        def d_host(groups):
            def host():
                for m, no in groups:
                    d_group(m, no)
            return host

        with nc.allow_low_precision(reason="bf16 within tolerance"):
            # Stage 1: head pairs 2-3 over all query blocks, B fillers inside
            idx = 0
            for n in range(4):
                for p in (2, 3):
                    for hh in range(2):
                        c_combo(n, p, hh,
                                filler_host(idx) if idx < len(fillers)
                                else None)
                        idx += 1

            # Stage 2: head pairs 0-1; D(n-1) groups ride in block n's shadow
            for n in range(4):
                dlist = ([(m, no) for m in range((n - 1) * 4, n * 4)
                          for no in range(2)] if n >= 1 else [])
                di = 0
                for p in (0, 1):
                    for hh in range(2):
                        c_combo(n, p, hh,
                                d_host(dlist[di:di + 2]) if di < len(dlist)
                                else None)
                        di += 2
            for m in range(12, 16):
                for no in range(2):
                    d_group(m, no)

    nc.finalize()
    return nc


def _get_program():
    global _PROG
    if _PROG is None:
        _PROG = _build_program()
    return _PROG


def _make_in_maps(inputs):
    from ml_dtypes import bfloat16
    X_Q = np.asarray(inputs["X_Q"], dtype=np.float32)
    X_KV = np.asarray(inputs["X_KV"], dtype=np.float32)
    mask = np.asarray(inputs["key_padding_mask"])
    W_Q = np.asarray(inputs["W_Q"], dtype=np.float32)
    W_K = np.asarray(inputs["W_K"], dtype=np.float32)
    W_V = np.asarray(inputs["W_V"], dtype=np.float32)
    W_O = np.asarray(inputs["W_O"], dtype=np.float32)
    in_maps = []
    for core in range(8):
        b, half = core // 2, core % 2
        idx = np.flatnonzero(~mask[b].astype(bool))
        nk = len(idx)
        assert nk <= SK, f"kept keys {nk} exceed padded SK={SK}"
        xkvc = np.zeros((SK, HID), dtype=np.float32)
        xkvc[:nk] = X_KV[b][idx]
        maskv = (np.arange(SK) < nk).astype(np.float32)

        blob = np.zeros((128, BLOBW), dtype=bfloat16)
        for c in range(8):
            blob[:, OWK + c * 512: OWK + (c + 1) * 512] = \
                W_K[c * 128:(c + 1) * 128, half * 512:(half + 1) * 512]
            blob[:, OXKV + c * SK: OXKV + (c + 1) * SK] = \
                xkvc[:, c * 128:(c + 1) * 128].T
            blob[:, OWV + c * 512: OWV + (c + 1) * 512] = \
                W_V[c * 128:(c + 1) * 128, half * 512:(half + 1) * 512]
            blob[:, OWQ + c * 512: OWQ + (c + 1) * 512] = \
                W_Q[c * 128:(c + 1) * 128, half * 512:(half + 1) * 512]
            for q in range(4):
                blob[:, OXQ + q * 4096 + c * 512: OXQ + q * 4096 + (c + 1) * 512] = \
                    X_Q[b][q * 512:(q + 1) * 512, c * 128:(c + 1) * 128].T
        for c in range(4):
            blob[:, OWO + c * 1024: OWO + (c + 1) * 1024] = \
                W_O[half * 512 + c * 128: half * 512 + (c + 1) * 128, :]
        blob[:, OMSK:OMSK + SKT] = maskv.reshape(SKT, 128).T
        in_maps.append({"blob": blob})
    return in_maps


def kernel(**inputs):
    nc = _get_program()
    in_maps = _make_in_maps(inputs)
    res = run_bass_kernel_spmd(nc, in_maps, core_ids=list(range(8)))
    out = np.empty((B, S, HID), dtype=np.float32)
    for b in range(B):
        out[b] = res.results[2 * b]["y"] + res.results[2 * b + 1]["y"]
    return out


# revision 3
# speedup vs baseline: 1.0092x; 1.0092x over previous
import sys

sys.path.insert(0, "/opt/trn_rl_repo")
import numpy as np
from contextlib import ExitStack

from concourse import bacc
import concourse.tile as tile
from concourse import mybir
from concourse.bass_utils import run_bass_kernel_spmd

fp32 = mybir.dt.float32
bf16 = mybir.dt.bfloat16
Exp = mybir.ActivationFunctionType.Exp

B, S, HID = 4, 2048, 1024
H, DK = 16, 64
SK = 1152          # compacted+padded key count (max kept keys = 1036)
SKT = SK // 128    # 9 sk tiles
NPAIR = 4          # head pairs per core (8 heads = half the 16)

# blob column offsets (bf16 words per partition), in DMA/first-use order
OWK = 0
OXKV = OWK + 4096
OWV = OXKV + SK * 8
OWQ = OWV + 4096
OXQ = OWQ + 4096
OWO = OXQ + 16384
OMSK = OWO + 4096
BLOBW = OMSK + 16

_PROG = None


def _build_program():
    nc = bacc.Bacc("TRN2", target_bir_lowering=False)

    blob = nc.dram_tensor("blob", [128, BLOBW], bf16, kind="ExternalInput")
    y = nc.dram_tensor("y", [S, HID], fp32, kind="ExternalOutput")

    # SBUF arena (bf16 word offsets per partition), everything resident:
    #   KT   [0..4608)       K^T pair-major: KT[p, pair*1152 + sk]
    #   QT   [4608..12800)   Q^T: QT[p, pair*2048 + sq]
    #   YPN  [12800..20992)  normalized attn out^T: YPN[p, pair*2048 + sq]
    #   VP   [20992..30208)  pair*2304 + st*256 + [Va(64)|ma(64)|Vb(64)|mb(64)]
    #   WK   [30208..34304)  c-major weight chunks
    #   XKV  [34304..43520)  c-major: [:, c*1152 + sk]
    #   WV   [43520..47616)
    #   WQ   [47616..51712)
    #   XQ   [51712..68096)  q-major quarters, c-major within: q*4096 + c*512
    #   WO   [68096..72192)
    arena = nc.alloc_sbuf_tensor("arena", [128, 72192], bf16)
    base = nc.lookup_mloc(arena).addr

    def at(name, words, off_words):
        return nc.alloc_sbuf_tensor_at(
            name, [128, words], bf16, offset=base + off_words * 2
        )

    KT = at("KT", 4608, 0)
    QT = at("QT", 8192, 4608)
    YPN = at("YPN", 8192, 12800)
    VP = at("VP", 9216, 20992)
    WKs = at("WKs", 4096, 30208)
    XKVs = at("XKVs", 9216, 34304)
    WVs = at("WVs", 4096, 43520)
    WQs = at("WQs", 4096, 47616)
    XQs = at("XQs", 16384, 51712)
    WOs = at("WOs", 4096, 68096)

    with tile.TileContext(nc) as tc, ExitStack() as ctx:
        misc = ctx.enter_context(tc.tile_pool(name="misc", bufs=1))
        pt_pool = ctx.enter_context(tc.tile_pool(name="ptp", bufs=3))
        ev_pool = ctx.enter_context(tc.tile_pool(name="evp", bufs=3))
        rc_pool = ctx.enter_context(tc.tile_pool(name="rcp", bufs=2))
        ps_e = ctx.enter_context(tc.tile_pool(name="pse", bufs=3, space="PSUM"))
        ps_y = ctx.enter_context(tc.tile_pool(name="psy", bufs=2, space="PSUM"))

        masktb = misc.tile([128, 16], bf16)
        nc.sync.dma_start(masktb[:], blob[:, OMSK:OMSK + 16])
        maskt = misc.tile([128, 16], fp32)
        nc.vector.tensor_copy(maskt[:], masktb[:])

        # input stream from the blob, sliced by first use and split across
        # both HWDGE rings (sync + scalar) so transfers overlap compute
        nc.sync.dma_start(WKs[:], blob[:, OWK:OWK + 4096])
        for c in range(8):
            nc.sync.dma_start(
                XKVs[:, c * SK: c * SK + 384],
                blob[:, OXKV + c * SK: OXKV + c * SK + 384])
        for c in range(8):
            nc.scalar.dma_start(
                XKVs[:, c * SK + 384: c * SK + 768],
                blob[:, OXKV + c * SK + 384: OXKV + c * SK + 768])
        nc.sync.dma_start(WVs[:], blob[:, OWV:OWV + 4096])
        for c in range(8):
            nc.sync.dma_start(
                XKVs[:, c * SK + 768:(c + 1) * SK],
                blob[:, OXKV + c * SK + 768: OXKV + (c + 1) * SK])
        nc.scalar.dma_start(WQs[:], blob[:, OWQ:OWQ + 4096])
        nc.sync.dma_start(XQs[:, 0:4096], blob[:, OXQ:OXQ + 4096])
        nc.scalar.dma_start(XQs[:, 4096:8192],
                            blob[:, OXQ + 4096:OXQ + 8192])
        nc.sync.dma_start(XQs[:, 8192:12288],
                          blob[:, OXQ + 8192:OXQ + 12288])
        nc.scalar.dma_start(XQs[:, 12288:16384],
                            blob[:, OXQ + 12288:OXQ + 16384])
        nc.scalar.dma_start(WOs[:], blob[:, OWO:OWO + 4096])

        ones = misc.tile([128, 64], bf16)
        nc.vector.memset(ones[:], 1.0)

        # ---- Phase A: K^T -> KT, V (masked) -> VP ----
        def ktr_chunk(p, off, w):
            pk = ps_y.tile([128, 512], fp32, name="psyt")
            for c in range(8):
                nc.tensor.matmul(
                    pk[:, 0:w],
                    WKs[:, c * 512 + p * 128: c * 512 + (p + 1) * 128],
                    XKVs[:, c * SK + off: c * SK + off + w],
                    start=(c == 0), stop=(c == 7))
            nc.vector.tensor_copy(
                KT[:, p * SK + off: p * SK + off + w], pk[:, 0:w])

        def v_tile(st):
            pv = ps_e.tile([128, 1024], fp32, name="pe")
            for c in range(8):
                nc.tensor.matmul(
                    pv[:, 0:512],
                    XKVs[:, c * SK + st * 128: c * SK + (st + 1) * 128],
                    WVs[:, c * 512:(c + 1) * 512],
                    start=(c == 0), stop=(c == 7))
            for h in range(8):
                o = (h // 2) * 2304 + st * 256 + (h % 2) * 128
                nc.vector.tensor_scalar_mul(
                    VP[:, o:o + 64], pv[:, h * 64:(h + 1) * 64],
                    maskt[:, st:st + 1])

        for p in range(NPAIR):
            ktr_chunk(p, 0, 384)
        for p in range(NPAIR):
            ktr_chunk(p, 384, 384)
        for st in range(6):
            v_tile(st)
        for p in range(NPAIR):
            ktr_chunk(p, 768, 384)
        for st in range(6, SKT):
            v_tile(st)

        # ---- Phase B seed: Q^T for query block q0, all pairs ----
        def b_group(q, p):
            pq = ps_y.tile([128, 512], fp32, name="psyt")
            for c in range(8):
                nc.tensor.matmul(
                    pq[:],
                    WQs[:, c * 512 + p * 128: c * 512 + (p + 1) * 128],
                    XQs[:, q * 4096 + c * 512: q * 4096 + (c + 1) * 512],
                    start=(c == 0), stop=(c == 7))
            nc.vector.tensor_copy(
                QT[:, p * 2048 + q * 512: p * 2048 + (q + 1) * 512], pq[:])

        for p in range(NPAIR):
            b_group(0, p)

        # init VP mask columns (denominator ones, masked); stage-1 pairs
        # (2,3) first so the first combos' A*V reads are ready in time
        for p in (2, 3, 0, 1):
            for hh in range(2):
                for st in range(SKT):
                    o = p * 2304 + st * 256 + hh * 128 + 64
                    nc.vector.tensor_scalar_mul(
                        VP[:, o:o + 64], ones[:], maskt[:, st:st + 1])

        # ---- Phases C (attention) and D (out-proj), B fillers inside C ----
        def d_group(m, no):
            pd = ps_y.tile([128, 512], fp32, name="psyt")
            for tt in range(NPAIR):
                nc.tensor.matmul(
                    pd[:],
                    YPN[:, tt * 2048 + m * 128: tt * 2048 + (m + 1) * 128],
                    WOs[:, tt * 1024 + no * 512: tt * 1024 + no * 512 + 512],
                    start=(tt == 0), stop=(tt == 3))
            ob = ev_pool.tile([128, 512], fp32)
            nc.vector.tensor_copy(ob[:], pd[:])
            nc.sync.dma_start(
                y[m * 128:(m + 1) * 128, no * 512: no * 512 + 512], ob[:])

        NST = (2, 2, 2, 2, 1)  # sk tiles per exp stage (9 total)

        def c_combo(n, p, hh, host=None):
            # software-pipelined: pe/exp run 3 stages ahead of the A*V
            # matmuls; hosted filler/out-proj work runs in the exp ramp so
            # PE never waits on ACT.  py is allocated AFTER host() so the
            # 2-buf psum rotation never recycles an open accumulator.
            qsl = QT[hh * 64:(hh + 1) * 64,
                     p * 2048 + n * 512: p * 2048 + n * 512 + 512]
            pts = []

            def emit_pe(k):
                pe = ps_e.tile([128, 1024], fp32)
                w = 512 * NST[k]
                for j in range(NST[k]):
                    st = 2 * k + j
                    nc.tensor.matmul(
                        pe[:, j * 512:(j + 1) * 512],
                        KT[hh * 64:(hh + 1) * 64,
                           p * SK + st * 128: p * SK + (st + 1) * 128],
                        qsl, start=True, stop=True,
                        tile_position=(hh * 64, 0))
                pt = pt_pool.tile([128, 1024], bf16)
                nc.scalar.activation(pt[:, 0:w], pe[:, 0:w], Exp, scale=0.125)
                pts.append(pt)

            def emit_py(k, py):
                for j in range(NST[k]):
                    st = 2 * k + j
                    nc.tensor.matmul(
                        py[:],
                        VP[:, p * 2304 + st * 256 + hh * 128:
                           p * 2304 + st * 256 + hh * 128 + 128],
                        pts[k][:, j * 512:(j + 1) * 512],
                        start=(st == 0), stop=(st == SKT - 1))

            for k in range(3):
                emit_pe(k)
            if host is not None:
                host()
            py = ps_y.tile([128, 512], fp32, name="psyt")
            for k in range(3, 5):
                emit_py(k - 3, py)
                emit_pe(k)
            for k in range(2, 5):
                emit_py(k, py)
            rc = rc_pool.tile([64, 512], fp32)
            nc.vector.reciprocal(rc[:], py[64:128, :])
            nc.vector.tensor_mul(
                YPN[hh * 64:(hh + 1) * 64,
                    p * 2048 + n * 512: p * 2048 + n * 512 + 512],
                py[0:64, :], rc[:])

        # B fillers for stage 1: one group behind each combo; each (q,p)
        # filler precedes the first stage-1 combo reading QT(q,p).
        fillers = [(1, 2), (1, 3), (2, 2), (2, 3), (1, 0), (1, 1),
                   (2, 0), (3, 2), (3, 3), (2, 1), (3, 0), (3, 1)]

        def filler_host(i):
            def host():
                fq, fp = fillers[i]
                b_group(fq, fp)
            return host

        def d_host(groups):
            def host():
                for m, no in groups:
                    d_group(m, no)
            return host

        with nc.allow_low_precision(reason="bf16 within tolerance"):
            # Stage 1: head pairs 2-3 over all query blocks, B fillers inside
            idx = 0
            for n in range(4):
                for p in (2, 3):
                    for hh in range(2):
                        c_combo(n, p, hh,
                                filler_host(idx) if idx < len(fillers)
                                else None)
                        idx += 1

            # Stage 2: head pairs 0-1; D(n-1) groups ride in block n's shadow
            for n in range(4):
                dlist = ([(m, no) for m in range((n - 1) * 4, n * 4)
                          for no in range(2)] if n >= 1 else [])
                di = 0
                for p in (0, 1):
                    for hh in range(2):
                        c_combo(n, p, hh,
                                d_host(dlist[di:di + 2]) if di < len(dlist)
                                else None)
                        di += 2
            for m in range(12, 16):
                for no in range(2):
                    d_group(m, no)

    nc.finalize()
    return nc


def _get_program():
    global _PROG
    if _PROG is None:
        _PROG = _build_program()
    return _PROG


def _make_in_maps(inputs):
    from ml_dtypes import bfloat16
    X_Q = np.asarray(inputs["X_Q"], dtype=np.float32)
    X_KV = np.asarray(inputs["X_KV"], dtype=np.float32)
    mask = np.asarray(inputs["key_padding_mask"])
    W_Q = np.asarray(inputs["W_Q"], dtype=np.float32)
    W_K = np.asarray(inputs["W_K"], dtype=np.float32)
    W_V = np.asarray(inputs["W_V"], dtype=np.float32)
    W_O = np.asarray(inputs["W_O"], dtype=np.float32)
    in_maps = []
    for core in range(8):
        b, half = core // 2, core % 2
        idx = np.flatnonzero(~mask[b].astype(bool))
        nk = len(idx)
        assert nk <= SK, f"kept keys {nk} exceed padded SK={SK}"
        xkvc = np.zeros((SK, HID), dtype=np.float32)
        xkvc[:nk] = X_KV[b][idx]
        maskv = (np.arange(SK) < nk).astype(np.float32)

        def wimg(W):
            return (W[:, half * 512:(half + 1) * 512]
                    .reshape(8, 128, 512).transpose(1, 0, 2).reshape(128, 4096))

        blob = np.zeros((128, BLOBW), dtype=bfloat16)
        blob[:, OWK:OWK + 4096] = wimg(W_K)
        blob[:, OXKV:OXKV + SK * 8] = \
            xkvc.reshape(SK, 8, 128).transpose(2, 1, 0).reshape(128, SK * 8)
        blob[:, OWV:OWV + 4096] = wimg(W_V)
        blob[:, OWQ:OWQ + 4096] = wimg(W_Q)
        blob[:, OXQ:OXQ + 16384] = \
            X_Q[b].reshape(4, 512, 8, 128).transpose(3, 0, 2, 1).reshape(128, 16384)
        blob[:, OWO:OWO + 4096] = \
            (W_O[half * 512:(half + 1) * 512]
             .reshape(4, 128, 1024).transpose(1, 0, 2).reshape(128, 4096))
        blob[:, OMSK:OMSK + SKT] = maskv.reshape(SKT, 128).T
        in_maps.append({"blob": blob})
    return in_maps


def kernel(**inputs):
    nc = _get_program()
    in_maps = _make_in_maps(inputs)
    res = run_bass_kernel_spmd(nc, in_maps, core_ids=list(range(8)))
    out = np.empty((B, S, HID), dtype=np.float32)
    for b in range(B):
        out[b] = res.results[2 * b]["y"] + res.results[2 * b + 1]["y"]
    return out
